# revision 40
# baseline (speedup 1.0000x reference)
"""Trainium2 Bass kernel for nn_ErrorAwareEdgeLoss.

reference:  cost[b,e] = sum_{p,q} P[b,i_e,p] * d_error[p,q] * P[b,j_e,q]
            result    = mean_{b,e} cost[b,e]

The edge pairs only enter through the count matrix
    C[l1,l2] = #edges e with (i_e,j_e) == (l1,l2),
and since d_error is symmetric the result collapses to
    result = <d_error, sum_b Q_b^T Cs Q_b> / (B*E),
with Q_b = P[b,:64,:] and Cs = (C + C^T)/2 (exact in fp8: half-integers).

Device work per core (256 batches, data-parallel over batch), all in fp8
(e4m3, host-packed as 64*Q — the softmax values are tiny so 64*Q stays
well inside [2^-6, 240] and the final host reduce divides by 64^2;
empirical rel err vs f64 reference ~2e-3, tolerance 2e-2):
  - warmup matmuls on a zeroed scratch tile start the PE before any DMA
    lands: the DVFS ramp (0.65->2.4GHz) needs ~4us of continuous PE
    activity, and an idle gap >1us resets it
  - group loads all go on the ONE sync HWDGE ring so they complete
    strictly in consumption order (small lead-in/tail groups, 16-pair
    steady groups; [128, pairs, 128] fp8 tiles, two batches per 128
    partitions)
  - Y = Cs @ Q via ONE blockdiag(Cs,Cs) matmul per 512-wide slab (the
    PE streams instructions serially, so fusing the two 64x64 quadrant
    matmuls halves the LDWEIGHTS traffic that paced Y); filler matmuls
    soak the deterministic lead-in stalls so the DVFS ramp never resets
  - cast Y (PSUM f32) to fp8 in SBUF as whole 512-col slabs alternating
    between DVE and ACT (fixed per-instruction overhead dominates smaller
    casts; half-slab splitting ran both engines at ~100% occupancy and
    gated the R matmuls)
  - R += Q^T Y with K=256 fp8 DoubleRow matmuls (two batch-pairs per
    instruction) accumulated in PSUM f32 — 2x PE throughput vs bf16 and
    half the LDWEIGHTS pressure per contracted column
  - write the per-core R (128x128 f32) to DRAM.
Host: R_total = sum_c R_c ;  result = <d_error, R_total> / (B*E*64^2).
"""

import sys

_TRN_REPO = "/opt/trn_rl_repo"
if _TRN_REPO not in sys.path:
    sys.path.insert(0, _TRN_REPO)

import numpy as np
import ml_dtypes

B, L, H = 2048, 64, 128     # batch, logical qubits, physical dim
E = 512                     # number of circuit edges
N_CORES = 8
BPC = B // N_CORES          # 256 batches per core
GROUP = 32                  # batches per DMA group
NGROUPS = BPC // GROUP      # 8
NPAIRS = BPC // 2           # 128 batch-pairs per core
PPG = GROUP // 2            # 16 pairs per steady-state group
GROUPS = [8, 8] + [16] * 6 + [8, 8]  # pairs per load: small lead-in groups
                            # start the PE sooner, small tail groups cut
                            # the trailing R run; sum must be NPAIRS
SLAB_PAIRS = 4              # pairs per Y-matmul slab (512 moving columns)
QSCALE = 64.0               # host-side scale before fp8 cast
WARMUP_MM = 26              # dummy matmuls to ramp the PE p-state: the
                            # DVFS reaches 2.4GHz ~9us after the PE first
                            # goes busy, so start it ASAP on scratch data
                            # while the first DMA is still in flight

_CACHE = {}


def _build():
    import concourse.tile as tile
    from concourse import bacc, mybir

    f32 = mybir.dt.float32
    fp8 = mybir.dt.float8e4

    nc = bacc.Bacc(None)
    # host-packed shard: pq[p, j, h] = 64*Q[2j + p//64, p%64, h] over the
    # core's 128 batch-pairs j — each group load is a plain 2D DMA with a
    # 2KB contiguous run per partition.
    pq = nc.dram_tensor("pq", [128, NPAIRS, H], fp8, kind="ExternalInput")
    # cs comes host-packed as blockdiag(Cs, Cs) so one matmul computes
    # both batch-halves' Y: one small DMA, one LDW per Y slab.
    cs = nc.dram_tensor("cs", [128, 128], fp8, kind="ExternalInput")
    r_out = nc.dram_tensor("r_out", [H, H], f32, kind="ExternalOutput")

    with tile.TileContext(nc) as tc:
        with (
            tc.tile_pool(name="singles", bufs=1) as singles,
            tc.tile_pool(name="qbfp", bufs=6) as qbf_pool,
            tc.tile_pool(name="ybfp", bufs=3) as ybf_pool,
            tc.tile_pool(name="yps", bufs=3, space="PSUM") as yps,
            tc.tile_pool(name="rps", bufs=1, space="PSUM") as rps,
            tc.tile_pool(name="wps", bufs=1, space="PSUM") as wps,
        ):
            # PE p-state warmup: matmuls on a zeroed scratch tile, no DMA
            # dependency, so the DVFS ramp starts as early as possible.
            # DVE does the tiny memset (gpsimd is busy until ~7us).
            scratch = singles.tile([128, 128], fp8)
            nc.vector.memset(scratch[:, :], 0)
            wm_psum = wps.tile([128, 128], f32)
            for _ in range(WARMUP_MM):
                nc.tensor.matmul(
                    wm_psum[:, :], lhsT=scratch[:, :], rhs=scratch[:, :],
                    start=True, stop=True, skip_group_check=True,
                )
            # blockdiag(Cs, Cs) stationary: a single full-height matmul
            # per slab replaces the two 64x64-quadrant matmuls — the PE
            # streams instructions serially anyway, and this halves the
            # LDWEIGHTS traffic that was pacing Y at 262ns/slab.  Loaded
            # on the SAME sync ring as the group loads, ahead of group 0
            # (the ring drains FIFO so the tiny load lands first).
            cs2 = singles.tile([128, 128], fp8)
            nc.sync.dma_start(out=cs2[:, :], in_=cs[:, :])

            r_psum = rps.tile([128, H], f32)

            # All group loads go on the ONE sync-queue ring: SDMA engines
            # drain a ring FIFO, so groups complete in consumption order
            # (two rings would round-robin and finish together, stalling
            # the PE on the second group).
            def load_group(p0, npairs):
                qbf = qbf_pool.tile([128, npairs, H], fp8)
                nc.sync.dma_start(
                    out=qbf[:, :, :], in_=pq[:, p0 : p0 + npairs, :]
                )
                return qbf

            _flags = {"first": True}

            def emit_y_slab2(qbf, ybf, s2):
                # two 512-col Y matmuls into one 2-bank PSUM tile, then
                # ONE 1024-col cast on an alternating engine — the ~240ns
                # fixed cast overhead had DVE/ACT back at ~95% occupancy
                # once the blockdiag Y sped the group period up to 1.43us.
                yy = yps.tile([128, 2 * SLAB_PAIRS * H], f32)
                for j in range(2):
                    sl = slice((2 * s2 + j) * SLAB_PAIRS, (2 * s2 + j + 1) * SLAB_PAIRS)
                    nc.tensor.matmul(
                        yy[:, j * SLAB_PAIRS * H : (j + 1) * SLAB_PAIRS * H],
                        lhsT=cs2[:, :], rhs=qbf[:, sl, :],
                        start=True, stop=True, skip_group_check=True,
                    )
                eng = nc.vector.tensor_copy if s2 % 2 == 0 else nc.scalar.copy
                eng(ybf[:, 2 * s2 * SLAB_PAIRS : 2 * (s2 + 1) * SLAB_PAIRS, :], yy[:, :])

            def emit_r_block(qbf, ybf, k, last):
                from concourse import mybir as mb

                first = _flags["first"]
                _flags["first"] = False
                nc.tensor.matmul(
                    r_psum[:, :],
                    lhsT=qbf[:, k : k + 2, :],
                    rhs=ybf[:, k : k + 2, :],
                    start=first, stop=last, skip_group_check=True,
                    perf_mode=mb.MatmulPerfMode.DoubleRow,
                )

            def emit_filler(n):
                # no-dependency matmuls that soak deterministic pipeline
                # stalls (PE outrunning DMA/casts during the lead-in)
                # without letting the DVFS ramp reset
                for _ in range(n):
                    nc.tensor.matmul(
                        wm_psum[:, :], lhsT=scratch[:, :], rhs=scratch[:, :],
                        start=True, stop=True, skip_group_check=True,
                    )

            # Software pipeline: R-matmuls run one group behind the
            # Y-matmuls so the PE never waits on the PSUM->SBUF casts.
            assert sum(GROUPS) == NPAIRS
            FILLERS = {0: 3, 1: 8, 2: 3}
            prev = None
            p0 = 0
            for gi, npairs in enumerate(GROUPS):
                qbf = load_group(p0, npairs)
                p0 += npairs
                ybf = ybf_pool.tile([128, npairs, H], fp8)
                for s2 in range(npairs // (2 * SLAB_PAIRS)):
                    emit_y_slab2(qbf, ybf, s2)
                if prev is not None:
                    pq_, py_, pn_ = prev
                    for k in range(0, pn_, 2):
                        emit_r_block(pq_, py_, k, last=False)
                emit_filler(FILLERS.get(gi, 0))
                prev = (qbf, ybf, npairs)
            pq_, py_, pn_ = prev
            for k in range(0, pn_, 2):
                emit_r_block(pq_, py_, k, last=(k == pn_ - 2))

            rsb = singles.tile([128, H], f32)
            nc.vector.tensor_copy(rsb[:, 0 : H // 2], r_psum[:, 0 : H // 2])
            nc.scalar.copy(rsb[:, H // 2 : H], r_psum[:, H // 2 : H])
            nc.sync.dma_start(out=r_out[:, :], in_=rsb[:, :])

    nc.compile()
    return nc


def get_nc():
    key = ("nc", "fp8")
    if key not in _CACHE:
        _CACHE[key] = _build()
    return _CACHE[key]


def make_count_matrix(circuit_edge_pairs):
    pairs = np.asarray(circuit_edge_pairs).astype(np.int64)
    C = np.zeros((L, L), np.float64)
    np.add.at(C, (pairs[:, 0], pairs[:, 1]), 1.0)
    Cs = (C + C.T) * 0.5
    cs8 = Cs.astype(ml_dtypes.float8_e4m3)
    bd = np.zeros((128, 128), ml_dtypes.float8_e4m3)
    bd[:L, :L] = cs8
    bd[L:, L:] = cs8
    return bd


def pack_shard(Q):
    """(256, 64, 128) f32 -> (128, 128, 128) fp8 with
    T[p, j, h] = 64*Q[2j + p//64, p%64, h]."""
    arr = (Q.reshape(NPAIRS, 2, L, H) * QSCALE).astype(ml_dtypes.float8_e4m3)
    return np.ascontiguousarray(arr.transpose(1, 2, 0, 3).reshape(128, NPAIRS, H))


def make_in_maps(P, circuit_edge_pairs):
    P = np.asarray(P)
    csb = make_count_matrix(circuit_edge_pairs)
    in_maps = []
    for c in range(N_CORES):
        shard = np.ascontiguousarray(
            P[c * BPC : (c + 1) * BPC, :L, :], dtype=np.float32
        )
        in_maps.append({"pq": pack_shard(shard), "cs": csb})
    return in_maps


def reduce_results(per_core_r, d_error):
    R = np.zeros((H, H), np.float64)
    for r in per_core_r:
        R += np.asarray(r).astype(np.float64)
    out = (np.asarray(d_error).astype(np.float64) * R).sum() / (B * E * QSCALE * QSCALE)
    return np.array(out, dtype=np.float32)


def run_spmd(P, circuit_edge_pairs, **kwargs):
    """Run on the 8 NeuronCores; returns (list of per-core R, BassKernelResults)."""
    from concourse.bass_utils import run_bass_kernel_spmd

    nc = get_nc()
    in_maps = make_in_maps(P, circuit_edge_pairs)
    res = run_bass_kernel_spmd(nc, in_maps, core_ids=list(range(N_CORES)), **kwargs)
    per_core_r = [res.results[c]["r_out"] for c in range(N_CORES)]
    return per_core_r, res


def kernel(P, d_error, circuit_edge_pairs, num_logical):
    assert int(num_logical) == L
    per_core_r, _ = run_spmd(P, circuit_edge_pairs)
    return reduce_results(per_core_r, d_error)


# revision 41
# speedup vs baseline: 1.0553x; 1.0553x over previous
"""Trainium2 Bass kernel for nn_ErrorAwareEdgeLoss.

reference:  cost[b,e] = sum_{p,q} P[b,i_e,p] * d_error[p,q] * P[b,j_e,q]
            result    = mean_{b,e} cost[b,e]

The edge pairs only enter through the count matrix
    C[l1,l2] = #edges e with (i_e,j_e) == (l1,l2),
and since d_error is symmetric the result collapses to
    result = <d_error, sum_b Q_b^T Cs Q_b> / (B*E),
with Q_b = P[b,:64,:] and Cs = (C + C^T)/2 (exact in fp8: half-integers).

Device work per core (256 batches, data-parallel over batch), all in fp8
(e4m3, host-packed as 64*Q — the softmax values are tiny so 64*Q stays
well inside [2^-6, 240] and the final host reduce divides by 64^2;
empirical rel err vs f64 reference ~2e-3, tolerance 2e-2):
  - warmup matmuls on a zeroed scratch tile start the PE before any DMA
    lands: the DVFS ramp (0.65->2.4GHz) needs ~4us of continuous PE
    activity, and an idle gap >1us resets it
  - group loads all go on the ONE sync HWDGE ring so they complete
    strictly in consumption order (small lead-in/tail groups, 16-pair
    steady groups; [128, pairs, 128] fp8 tiles, two batches per 128
    partitions)
  - Y = Cs @ Q via ONE blockdiag(Cs,Cs) matmul per 512-wide slab (the
    PE streams instructions serially, so fusing the two 64x64 quadrant
    matmuls halves the LDWEIGHTS traffic that paced Y); filler matmuls
    soak the deterministic lead-in stalls so the DVFS ramp never resets
  - cast Y (PSUM f32) to fp8 in SBUF as whole 512-col slabs alternating
    between DVE and ACT (fixed per-instruction overhead dominates smaller
    casts; half-slab splitting ran both engines at ~100% occupancy and
    gated the R matmuls)
  - R += Q^T Y with K=256 fp8 DoubleRow matmuls (two batch-pairs per
    instruction) accumulated in PSUM f32 — 2x PE throughput vs bf16 and
    half the LDWEIGHTS pressure per contracted column
  - write the per-core R (128x128 f32) to DRAM.
Host: R_total = sum_c R_c ;  result = <d_error, R_total> / (B*E*64^2).
"""

import sys

_TRN_REPO = "/opt/trn_rl_repo"
if _TRN_REPO not in sys.path:
    sys.path.insert(0, _TRN_REPO)

import numpy as np
import ml_dtypes

B, L, H = 2048, 64, 128     # batch, logical qubits, physical dim
E = 512                     # number of circuit edges
N_CORES = 8
BPC = B // N_CORES          # 256 batches per core
GROUP = 32                  # batches per DMA group
NGROUPS = BPC // GROUP      # 8
NPAIRS = BPC // 2           # 128 batch-pairs per core
PPG = GROUP // 2            # 16 pairs per steady-state group
GROUPS = [8, 8] + [16] * 6 + [8, 8]  # pairs per load: small lead-in groups
                            # start the PE sooner, small tail groups cut
                            # the trailing R run; sum must be NPAIRS
SLAB_PAIRS = 4              # pairs per Y-matmul slab (512 moving columns)
QSCALE = 64.0               # host-side scale before fp8 cast
WARMUP_MM = 26              # dummy matmuls to ramp the PE p-state: the
                            # DVFS reaches 2.4GHz ~9us after the PE first
                            # goes busy, so start it ASAP on scratch data
                            # while the first DMA is still in flight

_CACHE = {}


def _build():
    import concourse.tile as tile
    from concourse import bacc, mybir

    f32 = mybir.dt.float32
    fp8 = mybir.dt.float8e4

    nc = bacc.Bacc(None)
    # host-packed shard: pq[p, j, h] = 64*Q[2j + p//64, p%64, h] over the
    # core's 128 batch-pairs j — each group load is a plain 2D DMA with a
    # 2KB contiguous run per partition.
    pq = nc.dram_tensor("pq", [128, NPAIRS, H], fp8, kind="ExternalInput")
    # cs comes host-packed as blockdiag(Cs, Cs) so one matmul computes
    # both batch-halves' Y: one small DMA, one LDW per Y slab.
    cs = nc.dram_tensor("cs", [128, 128], fp8, kind="ExternalInput")
    r_out = nc.dram_tensor("r_out", [H, H], f32, kind="ExternalOutput")

    with tile.TileContext(nc) as tc:
        with (
            tc.tile_pool(name="singles", bufs=1) as singles,
            tc.tile_pool(name="qbfp", bufs=6) as qbf_pool,
            tc.tile_pool(name="ybfp", bufs=3) as ybf_pool,
            tc.tile_pool(name="yps", bufs=6, space="PSUM") as yps,
            tc.tile_pool(name="rps", bufs=1, space="PSUM") as rps,
            tc.tile_pool(name="wps", bufs=1, space="PSUM") as wps,
        ):
            # PE p-state warmup: matmuls on a zeroed scratch tile, no DMA
            # dependency, so the DVFS ramp starts as early as possible.
            # DVE does the tiny memset (gpsimd is busy until ~7us).
            scratch = singles.tile([128, 128], fp8)
            nc.vector.memset(scratch[:, :], 0)
            wm_psum = wps.tile([128, 128], f32)
            for _ in range(WARMUP_MM):
                nc.tensor.matmul(
                    wm_psum[:, :], lhsT=scratch[:, :], rhs=scratch[:, :],
                    start=True, stop=True, skip_group_check=True,
                )
            # blockdiag(Cs, Cs) stationary: a single full-height matmul
            # per slab replaces the two 64x64-quadrant matmuls — the PE
            # streams instructions serially anyway, and this halves the
            # LDWEIGHTS traffic that was pacing Y at 262ns/slab.  Loaded
            # on the SAME sync ring as the group loads, ahead of group 0
            # (the ring drains FIFO so the tiny load lands first).
            cs2 = singles.tile([128, 128], fp8)
            nc.sync.dma_start(out=cs2[:, :], in_=cs[:, :])

            r_psum = rps.tile([128, H], f32)

            # All group loads go on the ONE sync-queue ring: SDMA engines
            # drain a ring FIFO, so groups complete in consumption order
            # (two rings would round-robin and finish together, stalling
            # the PE on the second group).
            def load_group(p0, npairs):
                qbf = qbf_pool.tile([128, npairs, H], fp8)
                nc.sync.dma_start(
                    out=qbf[:, :, :], in_=pq[:, p0 : p0 + npairs, :]
                )
                return qbf

            _flags = {"first": True}

            def emit_y_slab(qbf, ybf, s):
                yy = yps.tile([128, SLAB_PAIRS * H], f32)
                sl = slice(s * SLAB_PAIRS, (s + 1) * SLAB_PAIRS)
                nc.tensor.matmul(
                    yy[:, :], lhsT=cs2[:, :], rhs=qbf[:, sl, :],
                    start=True, stop=True, skip_group_check=True,
                )
                # PSUM -> SBUF fp8 cast: whole slab on ONE engine,
                # alternating DVE/ACT per slab — the ~240ns fixed
                # instruction overhead dominates half-slab casts (which
                # ran both engines at ~100% occupancy and gated R), while
                # fusing TWO slabs per cast over-coarsens the pipeline
                # and delays R by ~1.7us end-to-end (measured both ways).
                eng = nc.vector.tensor_copy if s % 2 == 0 else nc.scalar.copy
                eng(ybf[:, s * SLAB_PAIRS : (s + 1) * SLAB_PAIRS, :], yy[:, :])

            def emit_r_block(qbf, ybf, k, last):
                from concourse import mybir as mb

                first = _flags["first"]
                _flags["first"] = False
                nc.tensor.matmul(
                    r_psum[:, :],
                    lhsT=qbf[:, k : k + 2, :],
                    rhs=ybf[:, k : k + 2, :],
                    start=first, stop=last, skip_group_check=True,
                    perf_mode=mb.MatmulPerfMode.DoubleRow,
                )

            def emit_filler(n):
                # no-dependency matmuls that soak deterministic pipeline
                # stalls (PE outrunning DMA/casts during the lead-in)
                # without letting the DVFS ramp reset
                for _ in range(n):
                    nc.tensor.matmul(
                        wm_psum[:, :], lhsT=scratch[:, :], rhs=scratch[:, :],
                        start=True, stop=True, skip_group_check=True,
                    )

            # Software pipeline: R-matmuls run one group behind the
            # Y-matmuls so the PE never waits on the PSUM->SBUF casts.
            assert sum(GROUPS) == NPAIRS
            FILLERS = {0: 3, 1: 8, 2: 3}
            prev = None
            p0 = 0
            for gi, npairs in enumerate(GROUPS):
                qbf = load_group(p0, npairs)
                p0 += npairs
                ybf = ybf_pool.tile([128, npairs, H], fp8)
                for s in range(npairs // SLAB_PAIRS):
                    emit_y_slab(qbf, ybf, s)
                if prev is not None:
                    pq_, py_, pn_ = prev
                    for k in range(0, pn_, 2):
                        emit_r_block(pq_, py_, k, last=False)
                emit_filler(FILLERS.get(gi, 0))
                prev = (qbf, ybf, npairs)
            pq_, py_, pn_ = prev
            for k in range(0, pn_, 2):
                emit_r_block(pq_, py_, k, last=(k == pn_ - 2))

            rsb = singles.tile([128, H], f32)
            nc.vector.tensor_copy(rsb[:, 0 : H // 2], r_psum[:, 0 : H // 2])
            nc.scalar.copy(rsb[:, H // 2 : H], r_psum[:, H // 2 : H])
            nc.sync.dma_start(out=r_out[:, :], in_=rsb[:, :])

    nc.compile()
    return nc


def get_nc():
    key = ("nc", "fp8")
    if key not in _CACHE:
        _CACHE[key] = _build()
    return _CACHE[key]


def make_count_matrix(circuit_edge_pairs):
    pairs = np.asarray(circuit_edge_pairs).astype(np.int64)
    C = np.zeros((L, L), np.float64)
    np.add.at(C, (pairs[:, 0], pairs[:, 1]), 1.0)
    Cs = (C + C.T) * 0.5
    cs8 = Cs.astype(ml_dtypes.float8_e4m3)
    bd = np.zeros((128, 128), ml_dtypes.float8_e4m3)
    bd[:L, :L] = cs8
    bd[L:, L:] = cs8
    return bd


def pack_shard(Q):
    """(256, 64, 128) f32 -> (128, 128, 128) fp8 with
    T[p, j, h] = 64*Q[2j + p//64, p%64, h]."""
    arr = (Q.reshape(NPAIRS, 2, L, H) * QSCALE).astype(ml_dtypes.float8_e4m3)
    return np.ascontiguousarray(arr.transpose(1, 2, 0, 3).reshape(128, NPAIRS, H))


def make_in_maps(P, circuit_edge_pairs):
    P = np.asarray(P)
    csb = make_count_matrix(circuit_edge_pairs)
    in_maps = []
    for c in range(N_CORES):
        shard = np.ascontiguousarray(
            P[c * BPC : (c + 1) * BPC, :L, :], dtype=np.float32
        )
        in_maps.append({"pq": pack_shard(shard), "cs": csb})
    return in_maps


def reduce_results(per_core_r, d_error):
    R = np.zeros((H, H), np.float64)
    for r in per_core_r:
        R += np.asarray(r).astype(np.float64)
    out = (np.asarray(d_error).astype(np.float64) * R).sum() / (B * E * QSCALE * QSCALE)
    return np.array(out, dtype=np.float32)


def run_spmd(P, circuit_edge_pairs, **kwargs):
    """Run on the 8 NeuronCores; returns (list of per-core R, BassKernelResults)."""
    from concourse.bass_utils import run_bass_kernel_spmd

    nc = get_nc()
    in_maps = make_in_maps(P, circuit_edge_pairs)
    res = run_bass_kernel_spmd(nc, in_maps, core_ids=list(range(N_CORES)), **kwargs)
    per_core_r = [res.results[c]["r_out"] for c in range(N_CORES)]
    return per_core_r, res


def kernel(P, d_error, circuit_edge_pairs, num_logical):
    assert int(num_logical) == L
    per_core_r, _ = run_spmd(P, circuit_edge_pairs)
    return reduce_results(per_core_r, d_error)


# revision 43
# speedup vs baseline: 1.0580x; 1.0026x over previous
"""Trainium2 Bass kernel for nn_ErrorAwareEdgeLoss.

reference:  cost[b,e] = sum_{p,q} P[b,i_e,p] * d_error[p,q] * P[b,j_e,q]
            result    = mean_{b,e} cost[b,e]

The edge pairs only enter through the count matrix
    C[l1,l2] = #edges e with (i_e,j_e) == (l1,l2),
and since d_error is symmetric the result collapses to
    result = <d_error, sum_b Q_b^T Cs Q_b> / (B*E),
with Q_b = P[b,:64,:] and Cs = (C + C^T)/2 (exact in fp8: half-integers).

Device work per core (256 batches, data-parallel over batch), all in fp8
(e4m3, host-packed as 64*Q — the softmax values are tiny so 64*Q stays
well inside [2^-6, 240] and the final host reduce divides by 64^2;
empirical rel err vs f64 reference ~2e-3, tolerance 2e-2):
  - warmup matmuls on a zeroed scratch tile start the PE before any DMA
    lands: the DVFS ramp (0.65->2.4GHz) needs ~4us of continuous PE
    activity, and an idle gap >1us resets it
  - group loads all go on the ONE sync HWDGE ring so they complete
    strictly in consumption order (small lead-in/tail groups, 16-pair
    steady groups; [128, pairs, 128] fp8 tiles, two batches per 128
    partitions)
  - Y = Cs @ Q via ONE blockdiag(Cs,Cs) matmul per 512-wide slab (the
    PE streams instructions serially, so fusing the two 64x64 quadrant
    matmuls halves the LDWEIGHTS traffic that paced Y); filler matmuls
    soak the deterministic lead-in stalls so the DVFS ramp never resets
  - cast Y (PSUM f32) to fp8 in SBUF as whole 512-col slabs alternating
    between DVE and ACT (fixed per-instruction overhead dominates smaller
    casts; half-slab splitting ran both engines at ~100% occupancy and
    gated the R matmuls)
  - R += Q^T Y with K=256 fp8 DoubleRow matmuls (two batch-pairs per
    instruction) accumulated in PSUM f32 — 2x PE throughput vs bf16 and
    half the LDWEIGHTS pressure per contracted column
  - write the per-core R (128x128 f32) to DRAM.
Host: R_total = sum_c R_c ;  result = <d_error, R_total> / (B*E*64^2).
"""

import sys

_TRN_REPO = "/opt/trn_rl_repo"
if _TRN_REPO not in sys.path:
    sys.path.insert(0, _TRN_REPO)

import numpy as np
import ml_dtypes

B, L, H = 2048, 64, 128     # batch, logical qubits, physical dim
E = 512                     # number of circuit edges
N_CORES = 8
BPC = B // N_CORES          # 256 batches per core
GROUP = 32                  # batches per DMA group
NGROUPS = BPC // GROUP      # 8
NPAIRS = BPC // 2           # 128 batch-pairs per core
PPG = GROUP // 2            # 16 pairs per steady-state group
GROUPS = [8, 8] + [16] * 6 + [8, 8]  # pairs per load: small lead-in groups
                            # start the PE sooner, small tail groups cut
                            # the trailing R run; sum must be NPAIRS
SLAB_PAIRS = 4              # pairs per Y-matmul slab (512 moving columns)
QSCALE = 64.0               # host-side scale before fp8 cast
WARMUP_MM = 26              # dummy matmuls to ramp the PE p-state: the
                            # DVFS reaches 2.4GHz ~9us after the PE first
                            # goes busy, so start it ASAP on scratch data
                            # while the first DMA is still in flight

_CACHE = {}


def _build():
    import concourse.tile as tile
    from concourse import bacc, mybir

    f32 = mybir.dt.float32
    fp8 = mybir.dt.float8e4

    nc = bacc.Bacc(None)
    # host-packed shard: pq[p, j, h] = 64*Q[2j + p//64, p%64, h] over the
    # core's 128 batch-pairs j — each group load is a plain 2D DMA with a
    # 2KB contiguous run per partition.
    pq = nc.dram_tensor("pq", [128, NPAIRS, H], fp8, kind="ExternalInput")
    # cs comes host-packed as blockdiag(Cs, Cs) so one matmul computes
    # both batch-halves' Y: one small DMA, one LDW per Y slab.
    cs = nc.dram_tensor("cs", [128, 128], fp8, kind="ExternalInput")
    r_out = nc.dram_tensor("r_out", [H, H], f32, kind="ExternalOutput")

    with tile.TileContext(nc) as tc:
        with (
            tc.tile_pool(name="singles", bufs=1) as singles,
            tc.tile_pool(name="qbfp", bufs=6) as qbf_pool,
            tc.tile_pool(name="ybfp", bufs=3) as ybf_pool,
            tc.tile_pool(name="yps", bufs=6, space="PSUM") as yps,
            tc.tile_pool(name="rps", bufs=1, space="PSUM") as rps,
            tc.tile_pool(name="wps", bufs=1, space="PSUM") as wps,
        ):
            # PE p-state warmup: matmuls on a zeroed scratch tile, no DMA
            # dependency, so the DVFS ramp starts as early as possible.
            # DVE does the tiny memset (gpsimd is busy until ~7us).
            scratch = singles.tile([128, 128], fp8)
            nc.vector.memset(scratch[:, :], 0)
            wm_psum = wps.tile([128, 128], f32)
            for _ in range(WARMUP_MM):
                nc.tensor.matmul(
                    wm_psum[:, :], lhsT=scratch[:, :], rhs=scratch[:, :],
                    start=True, stop=True, skip_group_check=True,
                )
            # blockdiag(Cs, Cs) stationary: a single full-height matmul
            # per slab replaces the two 64x64-quadrant matmuls — the PE
            # streams instructions serially anyway, and this halves the
            # LDWEIGHTS traffic that was pacing Y at 262ns/slab.  Loaded
            # on the SAME sync ring as the group loads, ahead of group 0
            # (the ring drains FIFO so the tiny load lands first).
            cs2 = singles.tile([128, 128], fp8)
            nc.sync.dma_start(out=cs2[:, :], in_=cs[:, :])

            r_psum = rps.tile([128, H], f32)

            # All group loads go on the ONE sync-queue ring: SDMA engines
            # drain a ring FIFO, so groups complete in consumption order
            # (two rings would round-robin and finish together, stalling
            # the PE on the second group).
            def load_group(p0, npairs):
                qbf = qbf_pool.tile([128, npairs, H], fp8)
                nc.sync.dma_start(
                    out=qbf[:, :, :], in_=pq[:, p0 : p0 + npairs, :]
                )
                return qbf

            _flags = {"first": True}

            def emit_y_slab(qbf, ybf, s):
                yy = yps.tile([128, SLAB_PAIRS * H], f32)
                sl = slice(s * SLAB_PAIRS, (s + 1) * SLAB_PAIRS)
                nc.tensor.matmul(
                    yy[:, :], lhsT=cs2[:, :], rhs=qbf[:, sl, :],
                    start=True, stop=True, skip_group_check=True,
                )
                # PSUM -> SBUF fp8 cast: whole slab on ONE engine,
                # alternating DVE/ACT per slab — the ~240ns fixed
                # instruction overhead dominates half-slab casts (which
                # ran both engines at ~100% occupancy and gated R), while
                # fusing TWO slabs per cast over-coarsens the pipeline
                # and delays R by ~1.7us end-to-end (measured both ways).
                eng = nc.vector.tensor_copy if s % 2 == 0 else nc.scalar.copy
                eng(ybf[:, s * SLAB_PAIRS : (s + 1) * SLAB_PAIRS, :], yy[:, :])

            def emit_r_block(qbf, ybf, k, last):
                from concourse import mybir as mb

                first = _flags["first"]
                _flags["first"] = False
                nc.tensor.matmul(
                    r_psum[:, :],
                    lhsT=qbf[:, k : k + 2, :],
                    rhs=ybf[:, k : k + 2, :],
                    start=first, stop=last, skip_group_check=True,
                    perf_mode=mb.MatmulPerfMode.DoubleRow,
                )

            def emit_filler(n):
                # no-dependency matmuls that soak deterministic pipeline
                # stalls (PE outrunning DMA/casts during the lead-in)
                # without letting the DVFS ramp reset
                for _ in range(n):
                    nc.tensor.matmul(
                        wm_psum[:, :], lhsT=scratch[:, :], rhs=scratch[:, :],
                        start=True, stop=True, skip_group_check=True,
                    )

            # Software pipeline: R-matmuls run one group behind the
            # Y-matmuls so the PE never waits on the PSUM->SBUF casts.
            assert sum(GROUPS) == NPAIRS
            FILLERS = {0: 3, 1: 8, 2: 3}
            prev = None
            p0 = 0
            for gi, npairs in enumerate(GROUPS):
                qbf = load_group(p0, npairs)
                p0 += npairs
                ybf = ybf_pool.tile([128, npairs, H], fp8)
                for s in range(npairs // SLAB_PAIRS):
                    emit_y_slab(qbf, ybf, s)
                if prev is not None:
                    pq_, py_, pn_ = prev
                    for k in range(0, pn_, 2):
                        emit_r_block(pq_, py_, k, last=False)
                emit_filler(FILLERS.get(gi, 0))
                prev = (qbf, ybf, npairs)
            pq_, py_, pn_ = prev
            for k in range(0, pn_, 2):
                emit_r_block(pq_, py_, k, last=(k == pn_ - 2))

            rsb = singles.tile([128, H], f32)
            nc.vector.tensor_copy(rsb[:, 0 : H // 2], r_psum[:, 0 : H // 2])
            nc.scalar.copy(rsb[:, H // 2 : H], r_psum[:, H // 2 : H])
            nc.sync.dma_start(out=r_out[:, :], in_=rsb[:, :])

    nc.compile()
    return nc


def get_nc():
    key = ("nc", "fp8")
    if key not in _CACHE:
        _CACHE[key] = _build()
    return _CACHE[key]


def make_count_matrix(circuit_edge_pairs):
    pairs = np.asarray(circuit_edge_pairs).astype(np.int64)
    C = np.zeros((L, L), np.float64)
    np.add.at(C, (pairs[:, 0], pairs[:, 1]), 1.0)
    Cs = (C + C.T) * 0.5
    cs8 = Cs.astype(ml_dtypes.float8_e4m3)
    bd = np.zeros((128, 128), ml_dtypes.float8_e4m3)
    bd[:L, :L] = cs8
    bd[L:, L:] = cs8
    return bd


def pack_shard(Q):
    """(256, 64, 128) f32 -> (128, 128, 128) fp8 with
    T[p, j, h] = 64*Q[2j + p//64, p%64, h]."""
    arr = (Q.reshape(NPAIRS, 2, L, H) * QSCALE).astype(ml_dtypes.float8_e4m3)
    return np.ascontiguousarray(arr.transpose(1, 2, 0, 3).reshape(128, NPAIRS, H))


def make_in_maps(P, circuit_edge_pairs):
    P = np.asarray(P)
    csb = make_count_matrix(circuit_edge_pairs)
    in_maps = []
    for c in range(N_CORES):
        shard = np.ascontiguousarray(
            P[c * BPC : (c + 1) * BPC, :L, :], dtype=np.float32
        )
        in_maps.append({"pq": pack_shard(shard), "cs": csb})
    return in_maps


def reduce_results(per_core_r, d_error):
    R = np.zeros((H, H), np.float64)
    for r in per_core_r:
        R += np.asarray(r).astype(np.float64)
    out = (np.asarray(d_error).astype(np.float64) * R).sum() / (B * E * QSCALE * QSCALE)
    return np.array(out, dtype=np.float32)


def run_spmd(P, circuit_edge_pairs, **kwargs):
    """Run on the 8 NeuronCores; returns (list of per-core R, BassKernelResults)."""
    from concourse.bass_utils import run_bass_kernel_spmd

    nc = get_nc()
    in_maps = make_in_maps(P, circuit_edge_pairs)
    res = run_bass_kernel_spmd(nc, in_maps, core_ids=list(range(N_CORES)), **kwargs)
    per_core_r = [res.results[c]["r_out"] for c in range(N_CORES)]
    return per_core_r, res


def kernel(P, d_error, circuit_edge_pairs, num_logical):
    assert int(num_logical) == L
    per_core_r, _ = run_spmd(P, circuit_edge_pairs)
    return reduce_results(per_core_r, d_error)
